# revision 1
# baseline (speedup 1.0000x reference)
"""Trainium2 Bass kernel for attention-pooling (AttLayer).

Computes, per batch row b:
    z   = x[b] @ W + bias            # [S, A]
    t   = tanh(z)
    sc  = t @ u                      # [S]
    e   = exp(sc) * mask[b]
    out = (x[b]^T @ e) / (sum(e) + 1e-7)   # [D]

Sharding: data-parallel over batch across 8 NeuronCores (8 rows each).

Optimizations:
- Masked positions contribute exactly zero (e is multiplied by the mask),
  so the host gathers only the unmasked positions per row and pads to a
  fixed compacted length S_c (a multiple of 128). This halves both DMA
  traffic and compute for ~50%-dense masks while computing the identical
  function (padding lanes are zeroed by the compacted mask).
- x is streamed twice in bf16: natural [S_c, D] layout for the weighted
  sum and transposed [D, S_c] layout for the score matmuls.
- Weight-major matmul loops amortize LDWEIGHTS: each W chunk is loaded
  once per batch and reused across all seq blocks.
- The final division by (sum(e) + EPS) happens on host after the gather.
"""

import math
import os
import numpy as np
import ml_dtypes

B, S, D, A = 64, 2048, 512, 256
NCORES = 8
BL = B // NCORES          # batches per core
EPS = 1e-7

_cache = {}
last_results = None       # BassKernelResults of the most recent run


def _blocks_of(S_c):
    """Split S_c into seq blocks of at most 512 (multiples of 128)."""
    out = []
    rem = S_c
    while rem > 0:
        blk = min(512, rem)
        out.append(blk)
        rem -= blk
    return out


def _build_bass(S_c):
    import concourse.mybir as mybir
    import concourse.tile as tile
    from concourse import bacc

    f32 = mybir.dt.float32
    bf16 = mybir.dt.bfloat16
    AF = mybir.ActivationFunctionType

    assert S_c % 128 == 0
    NCOL = S_c // 128          # 128-column groups
    blocks = _blocks_of(S_c)   # e.g. [512, 512, 256]
    NBLK = len(blocks)
    starts = [sum(blocks[:i]) for i in range(NBLK)]

    nc = bacc.Bacc()

    xt = nc.declare_dram_parameter("xt", [BL, D, S_c], bf16, isOutput=False)
    xn = nc.declare_dram_parameter("xn", [BL, S_c, D], bf16, isOutput=False)
    mt = nc.declare_dram_parameter("mt", [BL, 128, NCOL], bf16, isOutput=False)
    w = nc.declare_dram_parameter("w", [D, A], bf16, isOutput=False)
    u2 = nc.declare_dram_parameter("u2", [128, A // 128], bf16, isOutput=False)
    b2 = nc.declare_dram_parameter("b2", [128, A // 128], f32, isOutput=False)
    num = nc.declare_dram_parameter("num", [BL, 4, D], f32, isOutput=True)
    den = nc.declare_dram_parameter("den", [BL, 128, NBLK], f32, isOutput=True)

    NAC = A // 128
    NDC = D // 128

    with tile.TileContext(nc) as tc:
        with (
            tc.tile_pool(name="consts", bufs=1) as consts,
            tc.tile_pool(name="xtp", bufs=2) as xtp,
            tc.tile_pool(name="xnp", bufs=2) as xnp,
            tc.tile_pool(name="tts", bufs=8) as tts,
            tc.tile_pool(name="erowp", bufs=4) as erowp,
            tc.tile_pool(name="esbp", bufs=6) as esbp,
            tc.tile_pool(name="denp", bufs=2) as denp,
            tc.tile_pool(name="outp", bufs=2) as outp,
            tc.tile_pool(name="pt", bufs=5, space="PSUM") as pt,
            tc.tile_pool(name="psc", bufs=2, space="PSUM") as psc,
            tc.tile_pool(name="pacc", bufs=1, space="PSUM") as pacc,
        ):
            # --- constants, loaded once ---
            w_sb = consts.tile([128, NDC, A], bf16)  # [p, dchunk, a]
            nc.sync.dma_start(out=w_sb, in_=w.rearrange("(c p) a -> p c a", p=128))
            u_sb = consts.tile([128, NAC], bf16)
            nc.sync.dma_start(out=u_sb, in_=u2[:, :])
            b_sb = consts.tile([128, NAC], f32)
            nc.sync.dma_start(out=b_sb, in_=b2[:, :])
            mt_sb = consts.tile([128, BL, NCOL], bf16)
            nc.sync.dma_start(out=mt_sb, in_=mt.rearrange("b p r -> p b r"))

            for bi in range(BL):
                xt_t = xtp.tile([128, NDC, S_c], bf16)
                nc.sync.dma_start(
                    out=xt_t, in_=xt[bi].rearrange("(c p) s -> p c s", p=128)
                )
                xn_t = xnp.tile([128, NCOL, D], bf16)
                nc.sync.dma_start(
                    out=xn_t, in_=xn[bi].rearrange("(j p) d -> p j d", p=128)
                )

                den_t = denp.tile([128, NBLK], f32)

                # stage 1 (weight-major): z^T[a, s] = W^T @ x^T, then tanh
                tt_all = {}
                for ac in range(NAC):
                    ps_list = [
                        pt.tile([128, 512], f32, tag="pst", name=f"pst{i}")
                        for i in range(NBLK)
                    ]
                    for dc in range(NDC):
                        for blk in range(NBLK):
                            nc.tensor.matmul(
                                out=ps_list[blk][:, : blocks[blk]],
                                lhsT=w_sb[:, dc, ac * 128 : (ac + 1) * 128],
                                rhs=xt_t[:, dc, starts[blk] : starts[blk] + blocks[blk]],
                                start=(dc == 0),
                                stop=(dc == NDC - 1),
                            )
                    for blk in range(NBLK):
                        tt = tts.tile([128, 512], bf16, tag="tt")
                        nc.scalar.activation(
                            out=tt[:, : blocks[blk]],
                            in_=ps_list[blk][:, : blocks[blk]],
                            func=AF.Tanh,
                            bias=b_sb[:, ac : ac + 1],
                            scale=1.0,
                        )
                        tt_all[(ac, blk)] = tt

                # stage 2 + exp + e-transpose + mask, per block
                e_tiles = []
                for blk in range(NBLK):
                    ncols = blocks[blk] // 128
                    ps_sc = psc.tile([1, 512], f32, tag="psc")
                    for ac in range(NAC):
                        nc.tensor.matmul(
                            out=ps_sc[:, : blocks[blk]],
                            lhsT=u_sb[:, ac : ac + 1],
                            rhs=tt_all[(ac, blk)][:, : blocks[blk]],
                            start=(ac == 0),
                            stop=(ac == NAC - 1),
                        )
                    e_row = erowp.tile([1, 512], bf16, tag="erow")
                    nc.scalar.activation(
                        out=e_row[:, : blocks[blk]],
                        in_=ps_sc[:, : blocks[blk]],
                        func=AF.Exp,
                    )
                    # transpose e_row into columns via SBUF->SBUF DMA gather
                    e_raw = esbp.tile([128, 4], bf16, tag="eraw")
                    for j in range(ncols):
                        nc.sync.dma_start(
                            out=e_raw[:, j : j + 1],
                            in_=e_row[0:1, j * 128 : (j + 1) * 128],
                        )
                    e_sb = esbp.tile([128, 4], bf16, tag="esb")
                    c0 = starts[blk] // 128
                    nc.vector.tensor_mul(
                        out=e_sb[:, :ncols],
                        in0=e_raw[:, :ncols],
                        in1=mt_sb[:, bi, c0 : c0 + ncols],
                    )
                    nc.vector.reduce_sum(
                        out=den_t[:, blk : blk + 1],
                        in_=e_sb[:, :ncols],
                        axis=mybir.AxisListType.X,
                    )
                    e_tiles.append(e_sb)

                # weighted sum: 4 column-tiled concurrent M=1 matmuls; the
                # partial sums land on psum partitions 0/32/64/96 and the
                # host adds the 4 rows after the gather.
                ps_acc = pacc.tile([128, D], f32)
                nc.vector.memset(ps_acc, 0.0)
                last_k = {g: max(k for k in range(NCOL) if k % 4 == g) for g in range(min(4, NCOL))}
                k = 0
                for blk in range(NBLK):
                    ncols = blocks[blk] // 128
                    for j in range(ncols):
                        grp = k % 4
                        nc.tensor.matmul(
                            out=ps_acc[32 * grp : 32 * grp + 1, :],
                            lhsT=e_tiles[blk][:, j : j + 1],
                            rhs=xn_t[:, starts[blk] // 128 + j, :],
                            start=(k < 4),
                            stop=(k == last_k[grp]),
                            tile_position=(0, 32 * grp),
                        )
                        k += 1
                o_sb = outp.tile([128, D], f32)
                nc.vector.tensor_copy(out=o_sb, in_=ps_acc)
                for g in range(4):
                    nc.sync.dma_start(
                        out=num[bi, g : g + 1, :], in_=o_sb[32 * g : 32 * g + 1, :]
                    )
                nc.sync.dma_start(out=den[bi], in_=den_t)

    nc.finalize()
    return nc


def _get_nc(S_c):
    if S_c not in _cache:
        _cache[S_c] = _build_bass(S_c)
    return _cache[S_c]


def kernel(x, mask, W, b, u):
    global last_results
    from concourse.bass_utils import run_bass_kernel_spmd

    bf = ml_dtypes.bfloat16
    x = np.asarray(x, dtype=np.float32)
    mask = np.asarray(mask).astype(bool)

    counts = mask.sum(axis=1)
    maxc = int(counts.max())
    S_c = min(S, max(256, 128 * ((maxc + 127) // 128)))
    NCOL = S_c // 128
    NBLK = len(_blocks_of(S_c))

    # host-side compaction: gather unmasked positions, zero-pad to S_c
    xc = np.zeros((B, S_c, D), dtype=np.float32)
    for bi in range(B):
        idx = np.flatnonzero(mask[bi])
        xc[bi, : idx.size] = x[bi, idx]
    maskc = (np.arange(S_c)[None, :] < counts[:, None]).astype(np.float32)

    xn_h = xc.astype(bf)                                               # [B, S_c, D]
    xt_h = np.ascontiguousarray(xc.transpose(0, 2, 1)).astype(bf)      # [B, D, S_c]
    mt_h = np.ascontiguousarray(
        maskc.reshape(B, NCOL, 128).transpose(0, 2, 1)
    ).astype(bf)                                                       # [B, 128, NCOL]
    w_h = np.asarray(W, dtype=np.float32).astype(bf)                   # [D, A]
    u_h = np.ascontiguousarray(
        np.asarray(u, dtype=np.float32)[:, 0].reshape(A // 128, 128).T
    ).astype(bf)                                                       # [128, A/128]
    b_h = np.ascontiguousarray(
        np.asarray(b, dtype=np.float32).reshape(A // 128, 128).T
    ).astype(np.float32)                                               # [128, A/128]

    nc = _get_nc(S_c)
    in_maps = []
    for c in range(NCORES):
        sl = slice(c * BL, (c + 1) * BL)
        in_maps.append(
            {
                "xt": xt_h[sl],
                "xn": xn_h[sl],
                "mt": mt_h[sl],
                "w": w_h,
                "u2": u_h,
                "b2": b_h,
            }
        )

    try:
        res = run_bass_kernel_spmd(nc, in_maps, core_ids=list(range(NCORES)))
    except ModuleNotFoundError:
        # BASS_TRACE requested but the axon NTFF hook module is absent;
        # rerun without tracing.
        os.environ["BASS_NEVER_TRACE"] = "1"
        res = run_bass_kernel_spmd(nc, in_maps, core_ids=list(range(NCORES)))
    last_results = res

    num = np.concatenate([r["num"] for r in res.results], axis=0)      # [B, 4, D]
    den = np.concatenate([r["den"] for r in res.results], axis=0)      # [B, 128, NBLK]
    denom = den.sum(axis=(1, 2)).astype(np.float32) + np.float32(EPS)
    out = num.sum(axis=1) / denom[:, None]
    return out.astype(np.float32)



# revision 9
# speedup vs baseline: 1.6746x; 1.6746x over previous
"""Trainium2 Bass kernel for attention-pooling (AttLayer).

Computes, per batch row b:
    z   = x[b] @ W + bias            # [S, A]
    t   = tanh(z)
    sc  = t @ u                      # [S]
    e   = exp(sc) * mask[b]
    out = (x[b]^T @ e) / (sum(e) + 1e-7)   # [D]

Sharding: data-parallel over batch across 8 NeuronCores (8 rows each).

Design (v2):
- Host compacts unmasked positions per row (~50% dense mask) and zero-pads
  to S_c (multiple of 128). Padding rows of x are zero, so they contribute
  nothing to the numerator; the denominator is computed on host from the
  returned e row with the compacted mask. No mask work on device at all.
- x is streamed ONCE per batch in a transposed layout xt[p, dc*S_c+s] =
  x[s, dc*128+p], host-packed so each SBUF partition line is one contiguous
  9 KB DMA descriptor (128 descriptors/batch, ~1.2 MB per transfer).
- Stage 1 (weight-major): z^T[a, s] accumulated over 4 d-chunks, tanh with
  per-partition bias on ScalarE.
- Stage 2 uses a column-replicated u as lhsT (m=128 instead of m=1, same
  cycle count), so exp(score) lands broadcast across all 128 partitions.
- Weighted sum runs on VectorE as 4 fused tensor_tensor_reduce ops per
  batch: num[d] = sum_s xt[d, s] * e_bcast[d, s], directly from the xt
  layout. No second x stream, no transposes, no M=1 matmuls.
- Host: out = num / (sum(e * maskc) + EPS).
"""

import math
import os
import numpy as np
import ml_dtypes

B, S, D, A = 64, 2048, 512, 256
NCORES = 8
BL = B // NCORES          # batches per core
NDC = D // 128            # 4 d-chunks
NAC = A // 128            # 2 a-chunks
EPS = 1e-7

_cache = {}
last_results = None       # BassKernelResults of the most recent run


def _blocks_of(S_c):
    """Split S_c into seq blocks of at most 512 (multiples of 128)."""
    out = []
    rem = S_c
    while rem > 0:
        blk = min(512, rem)
        out.append(blk)
        rem -= blk
    return out


def _build_bass(S_c):
    import concourse.mybir as mybir
    import concourse.tile as tile
    from concourse import bacc

    f32 = mybir.dt.float32
    bf16 = mybir.dt.bfloat16
    AF = mybir.ActivationFunctionType
    ALU = mybir.AluOpType

    assert S_c % 128 == 0
    blocks = _blocks_of(S_c)   # e.g. [512, 512, 128]
    NBLK = len(blocks)
    starts = [sum(blocks[:i]) for i in range(NBLK)]

    nc = bacc.Bacc()

    xt = nc.declare_dram_parameter("xt", [BL, 128, NDC * S_c], bf16, isOutput=False)
    w2 = nc.declare_dram_parameter("w2", [128, NDC * A], bf16, isOutput=False)
    u2 = nc.declare_dram_parameter("u2", [128, NAC * 128], bf16, isOutput=False)
    b2 = nc.declare_dram_parameter("b2", [128, NAC], f32, isOutput=False)
    num = nc.declare_dram_parameter("num", [128, BL * NDC], f32, isOutput=True)
    eo = nc.declare_dram_parameter("eo", [BL, 1, S_c], f32, isOutput=True)

    with tile.TileContext(nc) as tc:
        with (
            tc.tile_pool(name="consts", bufs=1) as consts,
            tc.tile_pool(name="xtp", bufs=4) as xtp,
            tc.tile_pool(name="ttp", bufs=2) as ttp,
            tc.tile_pool(name="ebp", bufs=2) as ebp,
            tc.tile_pool(name="erp", bufs=2) as erp,
            tc.tile_pool(name="prodp", bufs=3) as prodp,
            tc.tile_pool(name="pt", bufs=5, space="PSUM") as pt,
            tc.tile_pool(name="psc", bufs=3, space="PSUM") as psc,
        ):
            # --- constants, loaded once ---
            w_sb = consts.tile([128, NDC * A], bf16)
            nc.sync.dma_start(out=w_sb, in_=w2[:, :])
            u_sb = consts.tile([128, NAC * 128], bf16)
            nc.sync.dma_start(out=u_sb, in_=u2[:, :])
            b_sb = consts.tile([128, NAC], f32)
            nc.sync.dma_start(out=b_sb, in_=b2[:, :])
            num_sb = consts.tile([128, BL * NDC], f32)

            for bi in range(BL):
                xt_t = xtp.tile([128, NDC * S_c], bf16)
                nc.sync.dma_start(out=xt_t, in_=xt[bi])

                # stage 1 (weight-major): z^T[a, s] = W^T @ x^T, then tanh
                tt = ttp.tile([128, NAC * S_c], bf16)
                for ac in range(NAC):
                    ps_list = [
                        pt.tile([128, 512], f32, tag="pst", name=f"pst{i}")
                        for i in range(NBLK)
                    ]
                    for dc in range(NDC):
                        lo = dc * A + ac * 128
                        for blk in range(NBLK):
                            nc.tensor.matmul(
                                out=ps_list[blk][:, : blocks[blk]],
                                lhsT=w_sb[:, lo : lo + 128],
                                rhs=xt_t[
                                    :,
                                    dc * S_c + starts[blk] : dc * S_c
                                    + starts[blk]
                                    + blocks[blk],
                                ],
                                start=(dc == 0),
                                stop=(dc == NDC - 1),
                            )
                    for blk in range(NBLK):
                        nc.scalar.activation(
                            out=tt[
                                :, ac * S_c + starts[blk] : ac * S_c + starts[blk] + blocks[blk]
                            ],
                            in_=ps_list[blk][:, : blocks[blk]],
                            func=AF.Tanh,
                            bias=b_sb[:, ac : ac + 1],
                            scale=1.0,
                        )

                # stage 2: score broadcast across partitions via replicated u,
                # then exp -> e_bcast [128, S_c]
                e_b = ebp.tile([128, S_c], bf16)
                sc_list = [
                    psc.tile([128, 512], f32, tag="psc", name=f"psc{i}")
                    for i in range(NBLK)
                ]
                for ac in range(NAC):
                    for blk in range(NBLK):
                        nc.tensor.matmul(
                            out=sc_list[blk][:, : blocks[blk]],
                            lhsT=u_sb[:, ac * 128 : (ac + 1) * 128],
                            rhs=tt[
                                :, ac * S_c + starts[blk] : ac * S_c + starts[blk] + blocks[blk]
                            ],
                            start=(ac == 0),
                            stop=(ac == NAC - 1),
                        )
                e32 = erp.tile([1, S_c], f32)
                for blk in range(NBLK):
                    nc.scalar.activation(
                        out=e_b[:, starts[blk] : starts[blk] + blocks[blk]],
                        in_=sc_list[blk][:, : blocks[blk]],
                        func=AF.Exp,
                    )
                    # f32 copy of row 0 for the host-side denominator
                    nc.scalar.activation(
                        out=e32[0:1, starts[blk] : starts[blk] + blocks[blk]],
                        in_=sc_list[blk][0:1, : blocks[blk]],
                        func=AF.Exp,
                    )
                nc.sync.dma_start(out=eo[bi], in_=e32[:, :])

                # weighted sum on VectorE: num[d] = sum_s xt[d, s] * e[s]
                for dc in range(NDC):
                    prod = prodp.tile([128, S_c], bf16, tag="prod")
                    col = bi * NDC + dc
                    nc.vector.tensor_mul(
                        out=prod,
                        in0=xt_t[:, dc * S_c : (dc + 1) * S_c],
                        in1=e_b,
                    )
                    nc.vector.reduce_sum(
                        out=num_sb[:, col : col + 1],
                        in_=prod,
                        axis=mybir.AxisListType.X,
                    )

            nc.sync.dma_start(out=num[:, :], in_=num_sb)

    nc.finalize()
    return nc


def _get_nc(S_c):
    if S_c not in _cache:
        _cache[S_c] = _build_bass(S_c)
    return _cache[S_c]


def kernel(x, mask, W, b, u):
    global last_results
    from concourse.bass_utils import run_bass_kernel_spmd

    bf = ml_dtypes.bfloat16
    x = np.asarray(x, dtype=np.float32)
    mask = np.asarray(mask).astype(bool)

    counts = mask.sum(axis=1)
    maxc = int(counts.max())
    S_c = min(S, max(256, 128 * ((maxc + 127) // 128)))

    # host-side compaction: gather unmasked positions, zero-pad to S_c
    xc = np.zeros((B, S_c, D), dtype=np.float32)
    for bi in range(B):
        idx = np.flatnonzero(mask[bi])
        xc[bi, : idx.size] = x[bi, idx]
    maskc = (np.arange(S_c)[None, :] < counts[:, None]).astype(np.float32)

    # xt_h[bi, p, dc*S_c + s] = xc[bi, s, dc*128 + p]; one contiguous
    # partition line per (bi, p) so each DMA is 128 big descriptors.
    xt_h = np.ascontiguousarray(
        xc.transpose(0, 2, 1)
        .reshape(B, NDC, 128, S_c)
        .transpose(0, 2, 1, 3)
        .reshape(B, 128, NDC * S_c)
    ).astype(bf)
    w2_h = np.ascontiguousarray(
        np.asarray(W, dtype=np.float32).reshape(NDC, 128, A).transpose(1, 0, 2).reshape(128, NDC * A)
    ).astype(bf)
    u_col = np.asarray(u, dtype=np.float32)[:, 0].reshape(NAC, 128).T  # [128, NAC]
    u2_h = np.ascontiguousarray(
        np.repeat(u_col[:, :, None], 128, axis=2).reshape(128, NAC * 128)
    ).astype(bf)
    b2_h = np.ascontiguousarray(
        np.asarray(b, dtype=np.float32).reshape(NAC, 128).T
    ).astype(np.float32)

    nc = _get_nc(S_c)
    in_maps = []
    for c in range(NCORES):
        sl = slice(c * BL, (c + 1) * BL)
        in_maps.append(
            {
                "xt": xt_h[sl],
                "w2": w2_h,
                "u2": u2_h,
                "b2": b2_h,
            }
        )

    try:
        res = run_bass_kernel_spmd(nc, in_maps, core_ids=list(range(NCORES)))
    except ModuleNotFoundError:
        # BASS_TRACE requested but the axon NTFF hook module is absent;
        # rerun without tracing.
        os.environ["BASS_NEVER_TRACE"] = "1"
        res = run_bass_kernel_spmd(nc, in_maps, core_ids=list(range(NCORES)))
    last_results = res

    out = np.empty((B, D), dtype=np.float32)
    for c in range(NCORES):
        sl = slice(c * BL, (c + 1) * BL)
        num_h = res.results[c]["num"]                    # [128, BL*NDC] f32
        e_h = res.results[c]["eo"].astype(np.float32)    # [BL, 1, S_c]
        num_bd = (
            num_h.reshape(128, BL, NDC).transpose(1, 2, 0).reshape(BL, D)
        )
        den = (e_h[:, 0, :] * maskc[sl]).sum(axis=1) + np.float32(EPS)
        out[sl] = num_bd / den[:, None]
    return out.astype(np.float32)


# revision 15
# speedup vs baseline: 1.7064x; 1.0190x over previous
"""Trainium2 Bass kernel for attention-pooling (AttLayer).

Computes, per batch row b:
    z   = x[b] @ W + bias            # [S, A]
    t   = tanh(z)
    sc  = t @ u                      # [S]
    e   = exp(sc) * mask[b]
    out = (x[b]^T @ e) / (sum(e) + 1e-7)   # [D]

Sharding: data-parallel over batch across 8 NeuronCores (8 rows each).

Design (v2):
- Host compacts unmasked positions per row (~50% dense mask) and zero-pads
  to S_c (multiple of 128). Padding rows of x are zero, so they contribute
  nothing to the numerator; the denominator is computed on host from the
  returned e row with the compacted mask. No mask work on device at all.
- x is streamed ONCE per batch in a transposed layout xt[p, dc*S_c+s] =
  x[s, dc*128+p], host-packed so each SBUF partition line is one contiguous
  9 KB DMA descriptor (128 descriptors/batch, ~1.2 MB per transfer).
- Stage 1 (weight-major): z^T[a, s] accumulated over 4 d-chunks, tanh with
  per-partition bias on ScalarE.
- Stage 2 uses a column-replicated u as lhsT (m=128 instead of m=1, same
  cycle count), so exp(score) lands broadcast across all 128 partitions.
- Weighted sum runs on VectorE as 4 fused tensor_tensor_reduce ops per
  batch: num[d] = sum_s xt[d, s] * e_bcast[d, s], directly from the xt
  layout. No second x stream, no transposes, no M=1 matmuls.
- Host: out = num / (sum(e * maskc) + EPS).
"""

import math
import os
import numpy as np
import ml_dtypes

B, S, D, A = 64, 2048, 512, 256
NCORES = 8
BL = B // NCORES          # batches per core
NDC = D // 128            # 4 d-chunks
NAC = A // 128            # 2 a-chunks
EPS = 1e-7

_cache = {}
last_results = None       # BassKernelResults of the most recent run


def _blocks_of(S_c):
    """Split S_c into seq blocks of at most 512 (multiples of 128)."""
    out = []
    rem = S_c
    while rem > 0:
        blk = min(512, rem)
        out.append(blk)
        rem -= blk
    return out


def _build_bass(S_c):
    import concourse.mybir as mybir
    import concourse.tile as tile
    from concourse import bacc
    from concourse.instruction_name_ordered_set import InstructionNameOrderedSet

    def dedup_ldweights(mms):
        """Matmuls in `mms` share the same lhsT: only the first self-loads
        weights; followers reuse the PE array state. A nosync dep chain
        pins their relative order on the Tensor engine."""
        for prev, mm in zip(mms, mms[1:]):
            mm.ins.ldweights = False
            deps = InstructionNameOrderedSet()
            deps.add(prev.ins.name)
            mm.ins.add_nosync_dependencies_from(deps)

    f32 = mybir.dt.float32
    bf16 = mybir.dt.bfloat16
    AF = mybir.ActivationFunctionType
    ALU = mybir.AluOpType

    assert S_c % 128 == 0
    blocks = _blocks_of(S_c)   # e.g. [512, 512, 128]
    NBLK = len(blocks)
    starts = [sum(blocks[:i]) for i in range(NBLK)]

    nc = bacc.Bacc()

    xt = nc.declare_dram_parameter("xt", [BL, 128, NDC * S_c], bf16, isOutput=False)
    w2 = nc.declare_dram_parameter("w2", [128, NDC * A], bf16, isOutput=False)
    u2 = nc.declare_dram_parameter("u2", [128, NAC * 128], bf16, isOutput=False)
    b2 = nc.declare_dram_parameter("b2", [128, NAC], f32, isOutput=False)
    num = nc.declare_dram_parameter("num", [128, BL * NDC], f32, isOutput=True)
    eo = nc.declare_dram_parameter("eo", [BL, 1, S_c], bf16, isOutput=True)

    with tile.TileContext(nc) as tc:
        with (
            tc.tile_pool(name="consts", bufs=1) as consts,
            tc.tile_pool(name="xtp", bufs=4) as xtp,
            tc.tile_pool(name="ttp", bufs=2) as ttp,
            tc.tile_pool(name="ebp", bufs=2) as ebp,
            tc.tile_pool(name="prodp", bufs=3) as prodp,
            tc.tile_pool(name="foldp", bufs=3) as foldp,
            tc.tile_pool(name="dumpp", bufs=2) as dumpp,
            tc.tile_pool(name="pt", bufs=5, space="PSUM") as pt,
            tc.tile_pool(name="psc", bufs=3, space="PSUM") as psc,
        ):
            # --- constants, loaded once ---
            w_sb = consts.tile([128, NDC * A], bf16)
            nc.sync.dma_start(out=w_sb, in_=w2[:, :])
            u_sb = consts.tile([128, NAC * 128], bf16)
            nc.sync.dma_start(out=u_sb, in_=u2[:, :])
            b_sb = consts.tile([128, NAC], f32)
            nc.sync.dma_start(out=b_sb, in_=b2[:, :])
            num_sb = consts.tile([128, BL * NDC], f32)

            for bi in range(BL):
                xt_t = xtp.tile([128, NDC * S_c], bf16)
                nc.sync.dma_start(out=xt_t, in_=xt[bi])

                # stage 1 (weight-major): z^T[a, s] = W^T @ x^T, then tanh
                tt = ttp.tile([128, NAC * S_c], bf16)
                for ac in range(NAC):
                    ps_list = [
                        pt.tile([128, 512], f32, tag="pst", name=f"pst{i}")
                        for i in range(NBLK)
                    ]
                    for dc in range(NDC):
                        lo = dc * A + ac * 128
                        mms = []
                        for blk in range(NBLK):
                            mms.append(
                                nc.tensor.matmul(
                                    out=ps_list[blk][:, : blocks[blk]],
                                    lhsT=w_sb[:, lo : lo + 128],
                                    rhs=xt_t[
                                        :,
                                        dc * S_c + starts[blk] : dc * S_c
                                        + starts[blk]
                                        + blocks[blk],
                                    ],
                                    start=(dc == 0),
                                    stop=(dc == NDC - 1),
                                )
                            )
                        dedup_ldweights(mms)
                    for blk in range(NBLK):
                        nc.scalar.activation(
                            out=tt[
                                :, ac * S_c + starts[blk] : ac * S_c + starts[blk] + blocks[blk]
                            ],
                            in_=ps_list[blk][:, : blocks[blk]],
                            func=AF.Tanh,
                            bias=b_sb[:, ac : ac + 1],
                            scale=1.0,
                        )

                # stage 2: score broadcast across partitions via replicated u,
                # then exp -> e_bcast [128, S_c]
                e_b = ebp.tile([128, S_c], bf16)
                sc_list = [
                    psc.tile([128, 512], f32, tag="psc", name=f"psc{i}")
                    for i in range(NBLK)
                ]
                for ac in range(NAC):
                    mms = []
                    for blk in range(NBLK):
                        mms.append(
                            nc.tensor.matmul(
                                out=sc_list[blk][:, : blocks[blk]],
                                lhsT=u_sb[:, ac * 128 : (ac + 1) * 128],
                                rhs=tt[
                                    :, ac * S_c + starts[blk] : ac * S_c + starts[blk] + blocks[blk]
                                ],
                                start=(ac == 0),
                                stop=(ac == NAC - 1),
                            )
                        )
                    dedup_ldweights(mms)
                for blk in range(NBLK):
                    nc.scalar.activation(
                        out=e_b[:, starts[blk] : starts[blk] + blocks[blk]],
                        in_=sc_list[blk][:, : blocks[blk]],
                        func=AF.Exp,
                    )
                nc.sync.dma_start(out=eo[bi], in_=e_b[0:1, :])

                # weighted sum: num[d] = sum_s xt[d, s] * e[s].
                # Multiply on VectorE (bf16 2x); the reduce runs folded on
                # VectorE for 3 chunks and as a Copy+accum on ScalarE for the
                # 4th, balancing engine load.
                H = S_c // 2
                for dc in range(NDC):
                    prod = prodp.tile([128, S_c], bf16, tag="prod")
                    col = bi * NDC + dc
                    nc.vector.tensor_mul(
                        out=prod,
                        in0=xt_t[:, dc * S_c : (dc + 1) * S_c],
                        in1=e_b,
                    )
                    if dc == NDC - 1:
                        dump = dumpp.tile([128, S_c], bf16, tag="dump")
                        nc.scalar.activation(
                            out=dump,
                            in_=prod,
                            func=AF.Copy,
                            accum_out=num_sb[:, col : col + 1],
                        )
                    else:
                        fold = foldp.tile([128, H], bf16, tag="fold")
                        nc.vector.tensor_add(
                            out=fold,
                            in0=prod[:, :H],
                            in1=prod[:, H:],
                        )
                        nc.vector.reduce_sum(
                            out=num_sb[:, col : col + 1],
                            in_=fold,
                            axis=mybir.AxisListType.X,
                        )

            nc.sync.dma_start(out=num[:, :], in_=num_sb)

    nc.finalize()
    return nc


def _get_nc(S_c):
    if S_c not in _cache:
        _cache[S_c] = _build_bass(S_c)
    return _cache[S_c]


def kernel(x, mask, W, b, u):
    global last_results
    from concourse.bass_utils import run_bass_kernel_spmd

    bf = ml_dtypes.bfloat16
    x = np.asarray(x, dtype=np.float32)
    mask = np.asarray(mask).astype(bool)

    counts = mask.sum(axis=1)
    maxc = int(counts.max())
    S_c = min(S, max(256, 128 * ((maxc + 127) // 128)))

    # host-side compaction: gather unmasked positions, zero-pad to S_c
    xc = np.zeros((B, S_c, D), dtype=np.float32)
    for bi in range(B):
        idx = np.flatnonzero(mask[bi])
        xc[bi, : idx.size] = x[bi, idx]
    maskc = (np.arange(S_c)[None, :] < counts[:, None]).astype(np.float32)

    # xt_h[bi, p, dc*S_c + s] = xc[bi, s, dc*128 + p]; one contiguous
    # partition line per (bi, p) so each DMA is 128 big descriptors.
    xt_h = np.ascontiguousarray(
        xc.transpose(0, 2, 1)
        .reshape(B, NDC, 128, S_c)
        .transpose(0, 2, 1, 3)
        .reshape(B, 128, NDC * S_c)
    ).astype(bf)
    w2_h = np.ascontiguousarray(
        np.asarray(W, dtype=np.float32).reshape(NDC, 128, A).transpose(1, 0, 2).reshape(128, NDC * A)
    ).astype(bf)
    u_col = np.asarray(u, dtype=np.float32)[:, 0].reshape(NAC, 128).T  # [128, NAC]
    u2_h = np.ascontiguousarray(
        np.repeat(u_col[:, :, None], 128, axis=2).reshape(128, NAC * 128)
    ).astype(bf)
    b2_h = np.ascontiguousarray(
        np.asarray(b, dtype=np.float32).reshape(NAC, 128).T
    ).astype(np.float32)

    nc = _get_nc(S_c)
    in_maps = []
    for c in range(NCORES):
        sl = slice(c * BL, (c + 1) * BL)
        in_maps.append(
            {
                "xt": xt_h[sl],
                "w2": w2_h,
                "u2": u2_h,
                "b2": b2_h,
            }
        )

    try:
        res = run_bass_kernel_spmd(nc, in_maps, core_ids=list(range(NCORES)))
    except ModuleNotFoundError:
        # BASS_TRACE requested but the axon NTFF hook module is absent;
        # rerun without tracing.
        os.environ["BASS_NEVER_TRACE"] = "1"
        res = run_bass_kernel_spmd(nc, in_maps, core_ids=list(range(NCORES)))
    last_results = res

    out = np.empty((B, D), dtype=np.float32)
    for c in range(NCORES):
        sl = slice(c * BL, (c + 1) * BL)
        num_h = res.results[c]["num"]                    # [128, BL*NDC] f32
        e_h = res.results[c]["eo"].astype(np.float32)    # [BL, 1, S_c]
        num_bd = (
            num_h.reshape(128, BL, NDC).transpose(1, 2, 0).reshape(BL, D)
        )
        den = (e_h[:, 0, :] * maskc[sl]).sum(axis=1) + np.float32(EPS)
        out[sl] = num_bd / den[:, None]
    return out.astype(np.float32)


# revision 22
# speedup vs baseline: 1.9695x; 1.1542x over previous
"""Trainium2 Bass kernel for attention-pooling (AttLayer).

Computes, per batch row b:
    z   = x[b] @ W + bias            # [S, A]
    t   = tanh(z)
    sc  = t @ u                      # [S]
    e   = exp(sc) * mask[b]
    out = (x[b]^T @ e) / (sum(e) + 1e-7)   # [D]

Sharding: data-parallel over batch across 8 NeuronCores (8 rows each).

Design (v2):
- Host compacts unmasked positions per row (~50% dense mask) and zero-pads
  to S_c (multiple of 128). Padding rows of x are zero, so they contribute
  nothing to the numerator; the denominator is computed on host from the
  returned e row with the compacted mask. No mask work on device at all.
- x is streamed ONCE per batch in a transposed layout xt[p, dc*S_c+s] =
  x[s, dc*128+p], host-packed so each SBUF partition line is one contiguous
  9 KB DMA descriptor (128 descriptors/batch, ~1.2 MB per transfer).
- Stage 1 (weight-major): z^T[a, s] accumulated over 4 d-chunks, tanh with
  per-partition bias on ScalarE.
- Stage 2 uses a column-replicated u as lhsT (m=128 instead of m=1, same
  cycle count), so exp(score) lands broadcast across all 128 partitions.
- Weighted sum runs on VectorE as 4 fused tensor_tensor_reduce ops per
  batch: num[d] = sum_s xt[d, s] * e_bcast[d, s], directly from the xt
  layout. No second x stream, no transposes, no M=1 matmuls.
- Host: out = num / (sum(e * maskc) + EPS).
"""

import math
import os
import numpy as np
import ml_dtypes

B, S, D, A = 64, 2048, 512, 256
NCORES = 8
BL = B // NCORES          # batches per core
NDC = D // 128            # 4 d-chunks
NAC = A // 128            # 2 a-chunks
EPS = 1e-7

_cache = {}
last_results = None       # BassKernelResults of the most recent run


def _blocks_of(S_c):
    """Split S_c into seq blocks of at most 512 (multiples of 128)."""
    out = []
    rem = S_c
    while rem > 0:
        blk = min(512, rem)
        out.append(blk)
        rem -= blk
    return out


def _build_bass(S_c):
    import concourse.mybir as mybir
    import concourse.tile as tile
    from concourse import bacc


    f32 = mybir.dt.float32
    bf16 = mybir.dt.bfloat16
    AF = mybir.ActivationFunctionType
    ALU = mybir.AluOpType

    assert S_c % 128 == 0
    blocks = _blocks_of(S_c)   # e.g. [512, 512, 128]
    NBLK = len(blocks)
    starts = [sum(blocks[:i]) for i in range(NBLK)]
    # main region (bank-aligned multiple of 512) + tail, for merged psum tiles
    S_main = (S_c // 512) * 512
    has_tail = S_main < S_c

    nc = bacc.Bacc()

    xt = nc.declare_dram_parameter("xt", [BL, 128, NDC * S_c], bf16, isOutput=False)
    w2 = nc.declare_dram_parameter("w2", [128, NDC * A], bf16, isOutput=False)
    u2 = nc.declare_dram_parameter("u2", [128, NAC * 128], bf16, isOutput=False)
    b2 = nc.declare_dram_parameter("b2", [128, NAC], f32, isOutput=False)
    num = nc.declare_dram_parameter("num", [128, BL * NDC], f32, isOutput=True)
    eo = nc.declare_dram_parameter("eo", [BL, 1, S_c], bf16, isOutput=True)

    with tile.TileContext(nc) as tc:
        with (
            tc.tile_pool(name="consts", bufs=1) as consts,
            tc.tile_pool(name="xtp", bufs=4) as xtp,
            tc.tile_pool(name="ttp", bufs=2) as ttp,
            tc.tile_pool(name="ebp", bufs=2) as ebp,
            tc.tile_pool(name="prodp", bufs=3) as prodp,
            tc.tile_pool(name="foldp", bufs=3) as foldp,
            tc.tile_pool(name="dumpp", bufs=2) as dumpp,
            tc.tile_pool(name="pt", bufs=2, space="PSUM") as pt,
            tc.tile_pool(name="ptt", bufs=1, space="PSUM") as ptt,
            tc.tile_pool(name="psc", bufs=1, space="PSUM") as psc,
        ):
            # --- constants, loaded once ---
            w_sb = consts.tile([128, NDC * A], bf16)
            nc.sync.dma_start(out=w_sb, in_=w2[:, :])
            u_sb = consts.tile([128, NAC * 128], bf16)
            nc.sync.dma_start(out=u_sb, in_=u2[:, :])
            b_sb = consts.tile([128, NAC], f32)
            nc.sync.dma_start(out=b_sb, in_=b2[:, :])
            num_sb = consts.tile([128, BL * NDC], f32)

            for bi in range(BL):
                xt_t = xtp.tile([128, NDC * S_c], bf16)
                nc.sync.dma_start(out=xt_t, in_=xt[bi])

                # stage 1 (weight-major): z^T[a, s] = W^T @ x^T, then tanh.
                # psum: one [128, S_main] tile (bank-aligned) + one tail tile,
                # so tanh runs as 2 big ScalarE instructions per a-chunk.
                tt = ttp.tile([128, NAC * S_c], bf16)
                for ac in range(NAC):
                    ps_big = pt.tile([128, S_main], f32, tag="pst", name="ps_big")
                    ps_tail = (
                        ptt.tile([128, 128], f32, tag="ptt", name="ps_tail")
                        if has_tail
                        else None
                    )
                    for dc in range(NDC):
                        lo = dc * A + ac * 128
                        for blk in range(NBLK):
                            st = starts[blk]
                            out_ps = (
                                ps_big[:, st : st + blocks[blk]]
                                if st < S_main
                                else ps_tail[:, : blocks[blk]]
                            )
                            nc.tensor.matmul(
                                out=out_ps,
                                lhsT=w_sb[:, lo : lo + 128],
                                rhs=xt_t[
                                    :,
                                    dc * S_c + st : dc * S_c + st + blocks[blk],
                                ],
                                start=(dc == 0),
                                stop=(dc == NDC - 1),
                            )
                    nc.scalar.activation(
                        out=tt[:, ac * S_c : ac * S_c + S_main],
                        in_=ps_big,
                        func=AF.Tanh,
                        bias=b_sb[:, ac : ac + 1],
                        scale=1.0,
                    )
                    if has_tail:
                        nc.scalar.activation(
                            out=tt[:, ac * S_c + S_main : ac * S_c + S_c],
                            in_=ps_tail[:, : S_c - S_main],
                            func=AF.Tanh,
                            bias=b_sb[:, ac : ac + 1],
                            scale=1.0,
                        )

                # stage 2: score broadcast across partitions via replicated u,
                # then exp -> e_bcast [128, S_c]
                e_b = ebp.tile([128, S_c], bf16)
                sc_ps = psc.tile([128, S_c], f32, tag="psc")
                for ac in range(NAC):
                    for blk in range(NBLK):
                        st = starts[blk]
                        nc.tensor.matmul(
                            out=sc_ps[:, st : st + blocks[blk]],
                            lhsT=u_sb[:, ac * 128 : (ac + 1) * 128],
                            rhs=tt[:, ac * S_c + st : ac * S_c + st + blocks[blk]],
                            start=(ac == 0),
                            stop=(ac == NAC - 1),
                        )
                nc.scalar.activation(out=e_b, in_=sc_ps, func=AF.Exp)
                nc.sync.dma_start(out=eo[bi], in_=e_b[0:1, :])

                # weighted sum: num[d] = sum_s xt[d, s] * e[s].
                # Multiply on VectorE (bf16 2x); the reduce runs folded on
                # VectorE for 3 chunks and as a Copy+accum on ScalarE for the
                # 4th, balancing engine load.
                H = S_c // 2
                for dc in range(NDC):
                    prod = prodp.tile([128, S_c], bf16, tag="prod")
                    col = bi * NDC + dc
                    nc.vector.tensor_mul(
                        out=prod,
                        in0=xt_t[:, dc * S_c : (dc + 1) * S_c],
                        in1=e_b,
                    )
                    if dc == NDC - 1:
                        dump = dumpp.tile([128, S_c], bf16, tag="dump")
                        nc.scalar.activation(
                            out=dump,
                            in_=prod,
                            func=AF.Copy,
                            accum_out=num_sb[:, col : col + 1],
                        )
                    else:
                        fold = foldp.tile([128, H], bf16, tag="fold")
                        nc.vector.tensor_add(
                            out=fold[:, : H // 2],
                            in0=prod[:, : H // 2],
                            in1=prod[:, H : H + H // 2],
                        )
                        nc.vector.tensor_add(
                            out=fold[:, H // 2 : H],
                            in0=prod[:, H // 2 : H],
                            in1=prod[:, H + H // 2 :],
                        )
                        nc.vector.tensor_add(
                            out=fold[:, : H // 2],
                            in0=fold[:, : H // 2],
                            in1=fold[:, H // 2 : H],
                        )
                        nc.vector.reduce_sum(
                            out=num_sb[:, col : col + 1],
                            in_=fold[:, : H // 2],
                            axis=mybir.AxisListType.X,
                        )

            nc.sync.dma_start(out=num[:, :], in_=num_sb)

    nc.finalize()
    return nc


def _get_nc(S_c):
    if S_c not in _cache:
        _cache[S_c] = _build_bass(S_c)
    return _cache[S_c]


def kernel(x, mask, W, b, u):
    global last_results
    from concourse.bass_utils import run_bass_kernel_spmd

    bf = ml_dtypes.bfloat16
    x = np.asarray(x, dtype=np.float32)
    mask = np.asarray(mask).astype(bool)

    counts = mask.sum(axis=1)
    maxc = int(counts.max())
    S_c = min(S, max(256, 128 * ((maxc + 127) // 128)))

    # host-side compaction: gather unmasked positions, zero-pad to S_c
    xc = np.zeros((B, S_c, D), dtype=np.float32)
    for bi in range(B):
        idx = np.flatnonzero(mask[bi])
        xc[bi, : idx.size] = x[bi, idx]
    maskc = (np.arange(S_c)[None, :] < counts[:, None]).astype(np.float32)

    # xt_h[bi, p, dc*S_c + s] = xc[bi, s, dc*128 + p]; one contiguous
    # partition line per (bi, p) so each DMA is 128 big descriptors.
    xt_h = np.ascontiguousarray(
        xc.transpose(0, 2, 1)
        .reshape(B, NDC, 128, S_c)
        .transpose(0, 2, 1, 3)
        .reshape(B, 128, NDC * S_c)
    ).astype(bf)
    w2_h = np.ascontiguousarray(
        np.asarray(W, dtype=np.float32).reshape(NDC, 128, A).transpose(1, 0, 2).reshape(128, NDC * A)
    ).astype(bf)
    u_col = np.asarray(u, dtype=np.float32)[:, 0].reshape(NAC, 128).T  # [128, NAC]
    u2_h = np.ascontiguousarray(
        np.repeat(u_col[:, :, None], 128, axis=2).reshape(128, NAC * 128)
    ).astype(bf)
    b2_h = np.ascontiguousarray(
        np.asarray(b, dtype=np.float32).reshape(NAC, 128).T
    ).astype(np.float32)

    nc = _get_nc(S_c)
    in_maps = []
    for c in range(NCORES):
        sl = slice(c * BL, (c + 1) * BL)
        in_maps.append(
            {
                "xt": xt_h[sl],
                "w2": w2_h,
                "u2": u2_h,
                "b2": b2_h,
            }
        )

    try:
        res = run_bass_kernel_spmd(nc, in_maps, core_ids=list(range(NCORES)))
    except ModuleNotFoundError:
        # BASS_TRACE requested but the axon NTFF hook module is absent;
        # rerun without tracing.
        os.environ["BASS_NEVER_TRACE"] = "1"
        res = run_bass_kernel_spmd(nc, in_maps, core_ids=list(range(NCORES)))
    last_results = res

    out = np.empty((B, D), dtype=np.float32)
    for c in range(NCORES):
        sl = slice(c * BL, (c + 1) * BL)
        num_h = res.results[c]["num"]                    # [128, BL*NDC] f32
        e_h = res.results[c]["eo"].astype(np.float32)    # [BL, 1, S_c]
        num_bd = (
            num_h.reshape(128, BL, NDC).transpose(1, 2, 0).reshape(BL, D)
        )
        den = (e_h[:, 0, :] * maskc[sl]).sum(axis=1) + np.float32(EPS)
        out[sl] = num_bd / den[:, None]
    return out.astype(np.float32)


# revision 26
# speedup vs baseline: 2.1239x; 1.0784x over previous
"""Trainium2 Bass kernel for attention-pooling (AttLayer).

Computes, per batch row b:
    z   = x[b] @ W + bias            # [S, A]
    t   = tanh(z)
    sc  = t @ u                      # [S]
    e   = exp(sc) * mask[b]
    out = (x[b]^T @ e) / (sum(e) + 1e-7)   # [D]

Sharding: data-parallel over batch across 8 NeuronCores (8 rows each).

Design (v5):
- Host compacts unmasked positions per row (~50% dense mask) and zero-pads.
  Padding rows of x are zero, so they contribute nothing to the numerator;
  the denominator is computed on host from the returned e row with the
  compacted mask. No mask work on device at all.
- Jagged slots: batches are sorted by unmasked count and distributed so
  slot j holds similar-count batches on every core. Each slot gets its own
  compacted length S_c[j] (64-multiple), so most slots run with 1024
  columns (two clean 512-wide matmul blocks, no tail) instead of the
  global max. One compile per slot-length tuple.
- x is streamed ONCE per batch in a transposed layout xt[p, dc*S_c+s] =
  x[s, dc*128+p], host-packed so each SBUF partition line is one
  contiguous DMA descriptor.
- Stage 1 (weight-major): z^T accumulated in a merged [128, <=1024] psum
  tile plus optional tail bank; tanh with per-partition bias runs as two
  big ScalarE instructions per a-chunk.
- Stage 2 uses a column-replicated u as lhsT (m=128, same cycle count as
  m=1), so exp(score) lands broadcast across all 128 partitions in one
  merged psum tile; a single Exp per batch.
- Weighted sum: tensor_mul on VectorE (bf16 2x) per d-chunk; reduction via
  a short fold tree on VectorE for 3 chunks and a Copy+accum_out on
  ScalarE for the 4th, balancing engine load.
- Host: out = num / (sum(e * maskc) + EPS), un-permuted.
"""

import math
import os
import numpy as np
import ml_dtypes

B, S, D, A = 64, 2048, 512, 256
NCORES = 8
BL = B // NCORES          # batches per core
NDC = D // 128            # 4 d-chunks
NAC = A // 128            # 2 a-chunks
EPS = 1e-7

_cache = {}
last_results = None       # BassKernelResults of the most recent run


def _blocks_of(S_c):
    """Split S_c into seq blocks of at most 512."""
    out = []
    rem = S_c
    while rem > 0:
        blk = min(512, rem)
        out.append(blk)
        rem -= blk
    return out


def _build_bass(sc_list):
    import concourse.mybir as mybir
    import concourse.tile as tile
    from concourse import bacc

    f32 = mybir.dt.float32
    bf16 = mybir.dt.bfloat16
    AF = mybir.ActivationFunctionType

    assert len(sc_list) == BL
    S_cmax = max(sc_list)
    assert all(sc % 64 == 0 for sc in sc_list)

    nc = bacc.Bacc()

    xt = nc.declare_dram_parameter("xt", [BL, 128, NDC * S_cmax], bf16, isOutput=False)
    w2 = nc.declare_dram_parameter("w2", [128, NDC * A], bf16, isOutput=False)
    u2 = nc.declare_dram_parameter("u2", [128, NAC * 128], bf16, isOutput=False)
    b2 = nc.declare_dram_parameter("b2", [128, NAC], f32, isOutput=False)
    num = nc.declare_dram_parameter("num", [128, BL * NDC], f32, isOutput=True)
    eo = nc.declare_dram_parameter("eo", [BL, 1, S_cmax], bf16, isOutput=True)

    with tile.TileContext(nc) as tc:
        with (
            tc.tile_pool(name="consts", bufs=1) as consts,
            tc.tile_pool(name="xtp", bufs=4) as xtp,
            tc.tile_pool(name="ttp", bufs=2) as ttp,
            tc.tile_pool(name="ebp", bufs=2) as ebp,
            tc.tile_pool(name="prodp", bufs=3) as prodp,
            tc.tile_pool(name="foldp", bufs=3) as foldp,
            tc.tile_pool(name="dumpp", bufs=2) as dumpp,
            tc.tile_pool(name="pt", bufs=2, space="PSUM") as pt,
            tc.tile_pool(name="ptt", bufs=1, space="PSUM") as ptt,
            tc.tile_pool(name="psc", bufs=1, space="PSUM") as psc,
        ):
            # --- constants, loaded once ---
            w_sb = consts.tile([128, NDC * A], bf16)
            nc.sync.dma_start(out=w_sb, in_=w2[:, :])
            u_sb = consts.tile([128, NAC * 128], bf16)
            nc.sync.dma_start(out=u_sb, in_=u2[:, :])
            b_sb = consts.tile([128, NAC], f32)
            nc.sync.dma_start(out=b_sb, in_=b2[:, :])
            num_sb = consts.tile([128, BL * NDC], f32)

            for bi in range(BL):
                S_c = sc_list[bi]
                blocks = _blocks_of(S_c)
                NBLK = len(blocks)
                starts = [sum(blocks[:i]) for i in range(NBLK)]
                S_main = min(S_c, 1024)
                has_tail = S_main < S_c

                xt_t = xtp.tile([128, NDC * S_cmax], bf16)
                nc.sync.dma_start(
                    out=xt_t[:, : NDC * S_c], in_=xt[bi][:, : NDC * S_c]
                )

                # stage 1 (weight-major): z^T[a, s] = W^T @ x^T, then tanh.
                tt = ttp.tile([128, NAC * S_cmax], bf16)
                for ac in range(NAC):
                    ps_big = pt.tile([128, 1024], f32, tag="pst", name="ps_big")
                    ps_tail = (
                        ptt.tile([128, 128], f32, tag="ptt", name="ps_tail")
                        if has_tail
                        else None
                    )
                    for dc in range(NDC):
                        lo = dc * A + ac * 128
                        for blk in range(NBLK):
                            st = starts[blk]
                            out_ps = (
                                ps_big[:, st : st + blocks[blk]]
                                if st < S_main
                                else ps_tail[:, : blocks[blk]]
                            )
                            nc.tensor.matmul(
                                out=out_ps,
                                lhsT=w_sb[:, lo : lo + 128],
                                rhs=xt_t[
                                    :,
                                    dc * S_c + st : dc * S_c + st + blocks[blk],
                                ],
                                start=(dc == 0),
                                stop=(dc == NDC - 1),
                            )
                    nc.scalar.activation(
                        out=tt[:, ac * S_c : ac * S_c + S_main],
                        in_=ps_big[:, :S_main],
                        func=AF.Tanh,
                        bias=b_sb[:, ac : ac + 1],
                        scale=1.0,
                    )
                    if has_tail:
                        nc.scalar.activation(
                            out=tt[:, ac * S_c + S_main : ac * S_c + S_c],
                            in_=ps_tail[:, : S_c - S_main],
                            func=AF.Tanh,
                            bias=b_sb[:, ac : ac + 1],
                            scale=1.0,
                        )

                # stage 2: score broadcast across partitions via replicated u,
                # one merged psum tile, one Exp -> e_bcast [128, S_c]
                e_b = ebp.tile([128, S_cmax], bf16)
                sc_ps = psc.tile([128, 1152], f32, tag="psc", name="sc_ps")
                for ac in range(NAC):
                    for blk in range(NBLK):
                        st = starts[blk]
                        nc.tensor.matmul(
                            out=sc_ps[:, st : st + blocks[blk]],
                            lhsT=u_sb[:, ac * 128 : (ac + 1) * 128],
                            rhs=tt[:, ac * S_c + st : ac * S_c + st + blocks[blk]],
                            start=(ac == 0),
                            stop=(ac == NAC - 1),
                        )
                nc.scalar.activation(
                    out=e_b[:, :S_c], in_=sc_ps[:, :S_c], func=AF.Exp
                )
                nc.sync.dma_start(out=eo[bi][:, :S_c], in_=e_b[0:1, :S_c])

                # weighted sum: num[d] = sum_s xt[d, s] * e[s].
                H = S_c // 2
                Q = H // 2
                for dc in range(NDC):
                    prod = prodp.tile([128, S_cmax], bf16, tag="prod")
                    col = bi * NDC + dc
                    nc.vector.tensor_mul(
                        out=prod[:, :S_c],
                        in0=xt_t[:, dc * S_c : dc * S_c + S_c],
                        in1=e_b[:, :S_c],
                    )
                    if dc == NDC - 1:
                        dump = dumpp.tile([128, S_cmax], bf16, tag="dump")
                        nc.scalar.activation(
                            out=dump[:, :S_c],
                            in_=prod[:, :S_c],
                            func=AF.Copy,
                            accum_out=num_sb[:, col : col + 1],
                        )
                    else:
                        fold = foldp.tile([128, S_cmax // 2], bf16, tag="fold")
                        nc.vector.tensor_add(
                            out=fold[:, :Q],
                            in0=prod[:, :Q],
                            in1=prod[:, H : H + Q],
                        )
                        nc.vector.tensor_add(
                            out=fold[:, Q:H],
                            in0=prod[:, Q:H],
                            in1=prod[:, H + Q : S_c],
                        )
                        nc.vector.tensor_add(
                            out=fold[:, :Q],
                            in0=fold[:, :Q],
                            in1=fold[:, Q:H],
                        )
                        nc.vector.reduce_sum(
                            out=num_sb[:, col : col + 1],
                            in_=fold[:, :Q],
                            axis=mybir.AxisListType.X,
                        )

            nc.sync.dma_start(out=num[:, :], in_=num_sb)

    nc.finalize()
    return nc


def _get_nc(sc_list):
    key = tuple(sc_list)
    if key not in _cache:
        _cache[key] = _build_bass(sc_list)
    return _cache[key]


def _prepare(x, mask, W, b, u):
    bf = ml_dtypes.bfloat16
    x = np.asarray(x, dtype=np.float32)
    mask = np.asarray(mask).astype(bool)

    counts = mask.sum(axis=1)

    # sort batches by count (desc); batch perm[j*NCORES + c] -> core c, slot j.
    # Slot j then needs only the max count within its band, rounded to 64.
    perm = np.argsort(-counts, kind="stable")
    sc_list = []
    for j in range(BL):
        band = counts[perm[j * NCORES : (j + 1) * NCORES]]
        mx = int(band.max())
        sc_list.append(min(S, max(256, 64 * ((mx + 63) // 64))))
    S_cmax = max(sc_list)

    # host-side compaction into the jagged packed layout:
    # xt_h[bi_slot, p, dc*S_c[j] + s] = x[batch, s_unmasked, dc*128 + p]
    xt_h = np.zeros((B, 128, NDC * S_cmax), dtype=bf)
    maskc = np.zeros((B, S_cmax), dtype=np.float32)
    for j in range(BL):
        S_c = sc_list[j]
        for c in range(NCORES):
            bidx = int(perm[j * NCORES + c])
            idx = np.flatnonzero(mask[bidx])
            xcb = np.zeros((S_c, D), dtype=np.float32)
            xcb[: idx.size] = x[bidx, idx]
            # [S_c, D] -> [128, NDC*S_c] with layout p, dc*S_c + s
            packed = (
                xcb.T.reshape(NDC, 128, S_c).transpose(1, 0, 2).reshape(128, NDC * S_c)
            )
            xt_h[c * BL + j, :, : NDC * S_c] = packed.astype(bf)
            maskc[c * BL + j, : idx.size] = 1.0

    w2_h = np.ascontiguousarray(
        np.asarray(W, dtype=np.float32).reshape(NDC, 128, A).transpose(1, 0, 2).reshape(128, NDC * A)
    ).astype(bf)
    u_col = np.asarray(u, dtype=np.float32)[:, 0].reshape(NAC, 128).T  # [128, NAC]
    u2_h = np.ascontiguousarray(
        np.repeat(u_col[:, :, None], 128, axis=2).reshape(128, NAC * 128)
    ).astype(bf)
    b2_h = np.ascontiguousarray(
        np.asarray(b, dtype=np.float32).reshape(NAC, 128).T
    ).astype(np.float32)
    return sc_list, perm, xt_h, maskc, w2_h, u2_h, b2_h


def kernel(x, mask, W, b, u):
    global last_results
    from concourse.bass_utils import run_bass_kernel_spmd

    sc_list, perm, xt_h, maskc, w2_h, u2_h, b2_h = _prepare(x, mask, W, b, u)
    nc = _get_nc(sc_list)
    in_maps = []
    for c in range(NCORES):
        sl = slice(c * BL, (c + 1) * BL)
        in_maps.append(
            {
                "xt": xt_h[sl],
                "w2": w2_h,
                "u2": u2_h,
                "b2": b2_h,
            }
        )

    try:
        res = run_bass_kernel_spmd(nc, in_maps, core_ids=list(range(NCORES)))
    except ModuleNotFoundError:
        # BASS_TRACE requested but the axon NTFF hook module is absent;
        # rerun without tracing.
        os.environ["BASS_NEVER_TRACE"] = "1"
        res = run_bass_kernel_spmd(nc, in_maps, core_ids=list(range(NCORES)))
    last_results = res

    out = np.empty((B, D), dtype=np.float32)
    for c in range(NCORES):
        num_h = res.results[c]["num"]                    # [128, BL*NDC] f32
        e_h = res.results[c]["eo"].astype(np.float32)    # [BL, 1, S_cmax]
        num_bd = (
            num_h.reshape(128, BL, NDC).transpose(1, 2, 0).reshape(BL, D)
        )
        for j in range(BL):
            bidx = int(perm[j * NCORES + c])
            sc = sc_list[j]
            den = (e_h[j, 0, :sc] * maskc[c * BL + j, :sc]).sum() + np.float32(EPS)
            out[bidx] = num_bd[j] / den
    return out.astype(np.float32)


# revision 27
# speedup vs baseline: 2.3088x; 1.0871x over previous
"""Trainium2 Bass kernel for attention-pooling (AttLayer).

Computes, per batch row b:
    z   = x[b] @ W + bias            # [S, A]
    t   = tanh(z)
    sc  = t @ u                      # [S]
    e   = exp(sc) * mask[b]
    out = (x[b]^T @ e) / (sum(e) + 1e-7)   # [D]

Sharding: data-parallel over batch across 8 NeuronCores (8 rows each).

Design (v5):
- Host compacts unmasked positions per row (~50% dense mask) and zero-pads.
  Padding rows of x are zero, so they contribute nothing to the numerator;
  the denominator is computed on host from the returned e row with the
  compacted mask. No mask work on device at all.
- Jagged slots: batches are sorted by unmasked count and distributed so
  slot j holds similar-count batches on every core. Each slot gets its own
  compacted length S_c[j] (64-multiple), so most slots run with 1024
  columns (two clean 512-wide matmul blocks, no tail) instead of the
  global max. One compile per slot-length tuple.
- x is streamed ONCE per batch in a transposed layout xt[p, dc*S_c+s] =
  x[s, dc*128+p], host-packed so each SBUF partition line is one
  contiguous DMA descriptor.
- Stage 1 (weight-major): z^T accumulated in a merged [128, <=1024] psum
  tile plus optional tail bank; tanh with per-partition bias runs as two
  big ScalarE instructions per a-chunk.
- Stage 2 uses a column-replicated u as lhsT (m=128, same cycle count as
  m=1), so exp(score) lands broadcast across all 128 partitions in one
  merged psum tile; a single Exp per batch.
- Weighted sum: tensor_mul on VectorE (bf16 2x) per d-chunk; reduction via
  a short fold tree on VectorE for 3 chunks and a Copy+accum_out on
  ScalarE for the 4th, balancing engine load.
- Host: out = num / (sum(e * maskc) + EPS), un-permuted.
"""

import math
import os
import numpy as np
import ml_dtypes

B, S, D, A = 64, 2048, 512, 256
NCORES = 8
BL = B // NCORES          # batches per core
NDC = D // 128            # 4 d-chunks
NAC = A // 128            # 2 a-chunks
EPS = 1e-7

_cache = {}
last_results = None       # BassKernelResults of the most recent run


def _blocks_of(S_c):
    """Split S_c into seq blocks of at most 512."""
    out = []
    rem = S_c
    while rem > 0:
        blk = min(512, rem)
        out.append(blk)
        rem -= blk
    return out


def _build_bass(sc_list):
    import concourse.mybir as mybir
    import concourse.tile as tile
    from concourse import bacc

    f32 = mybir.dt.float32
    bf16 = mybir.dt.bfloat16
    AF = mybir.ActivationFunctionType

    assert len(sc_list) == BL
    S_cmax = max(sc_list)
    assert all(sc % 64 == 0 for sc in sc_list)

    nc = bacc.Bacc()

    xt = nc.declare_dram_parameter("xt", [BL, 128, NDC * S_cmax], bf16, isOutput=False)
    w2 = nc.declare_dram_parameter("w2", [128, NDC * A], bf16, isOutput=False)
    u2 = nc.declare_dram_parameter("u2", [128, NAC * 128], bf16, isOutput=False)
    b2 = nc.declare_dram_parameter("b2", [128, NAC], f32, isOutput=False)
    num = nc.declare_dram_parameter("num", [128, BL * NDC], f32, isOutput=True)
    eo = nc.declare_dram_parameter("eo", [BL, 1, S_cmax], bf16, isOutput=True)

    with tile.TileContext(nc) as tc:
        with (
            tc.tile_pool(name="consts", bufs=1) as consts,
            tc.tile_pool(name="xtp", bufs=4) as xtp,
            tc.tile_pool(name="ttp", bufs=2) as ttp,
            tc.tile_pool(name="ebp", bufs=2) as ebp,
            tc.tile_pool(name="prodp", bufs=3) as prodp,
            tc.tile_pool(name="foldp", bufs=3) as foldp,
            tc.tile_pool(name="dumpp", bufs=2) as dumpp,
            tc.tile_pool(name="pt", bufs=2, space="PSUM") as pt,
            tc.tile_pool(name="ptt", bufs=1, space="PSUM") as ptt,
            tc.tile_pool(name="psc", bufs=1, space="PSUM") as psc,
        ):
            # --- constants, loaded once ---
            w_sb = consts.tile([128, NDC * A], bf16)
            nc.sync.dma_start(out=w_sb, in_=w2[:, :])
            u_sb = consts.tile([128, NAC * 128], bf16)
            nc.sync.dma_start(out=u_sb, in_=u2[:, :])
            b_sb = consts.tile([128, NAC], f32)
            nc.sync.dma_start(out=b_sb, in_=b2[:, :])
            num_sb = consts.tile([128, BL * NDC], f32)

            for bi in range(BL):
                S_c = sc_list[bi]
                blocks = _blocks_of(S_c)
                NBLK = len(blocks)
                starts = [sum(blocks[:i]) for i in range(NBLK)]
                S_main = min(S_c, 1024)
                has_tail = S_main < S_c

                xt_t = xtp.tile([128, NDC * S_cmax], bf16)
                nc.sync.dma_start(
                    out=xt_t[:, : NDC * S_c], in_=xt[bi][:, : NDC * S_c]
                )

                # stage 1 (weight-major): z^T[a, s] = W^T @ x^T, then tanh.
                tt = ttp.tile([128, NAC * S_cmax], bf16)
                for ac in range(NAC):
                    ps_big = pt.tile([128, 1024], f32, tag="pst", name="ps_big")
                    ps_tail = (
                        ptt.tile([128, 128], f32, tag="ptt", name="ps_tail")
                        if has_tail
                        else None
                    )
                    for dc in range(NDC):
                        lo = dc * A + ac * 128
                        for blk in range(NBLK):
                            st = starts[blk]
                            out_ps = (
                                ps_big[:, st : st + blocks[blk]]
                                if st < S_main
                                else ps_tail[:, : blocks[blk]]
                            )
                            nc.tensor.matmul(
                                out=out_ps,
                                lhsT=w_sb[:, lo : lo + 128],
                                rhs=xt_t[
                                    :,
                                    dc * S_c + st : dc * S_c + st + blocks[blk],
                                ],
                                start=(dc == 0),
                                stop=(dc == NDC - 1),
                            )
                    nc.scalar.activation(
                        out=tt[:, ac * S_c : ac * S_c + S_main],
                        in_=ps_big[:, :S_main],
                        func=AF.Tanh,
                        bias=b_sb[:, ac : ac + 1],
                        scale=1.0,
                    )
                    if has_tail:
                        nc.scalar.activation(
                            out=tt[:, ac * S_c + S_main : ac * S_c + S_c],
                            in_=ps_tail[:, : S_c - S_main],
                            func=AF.Tanh,
                            bias=b_sb[:, ac : ac + 1],
                            scale=1.0,
                        )

                # stage 2: score broadcast across partitions via replicated u,
                # one merged psum tile, one Exp -> e_bcast [128, S_c]
                e_b = ebp.tile([128, S_cmax], bf16)
                sc_ps = psc.tile([128, 1152], f32, tag="psc", name="sc_ps")
                for ac in range(NAC):
                    for blk in range(NBLK):
                        st = starts[blk]
                        nc.tensor.matmul(
                            out=sc_ps[:, st : st + blocks[blk]],
                            lhsT=u_sb[:, ac * 128 : (ac + 1) * 128],
                            rhs=tt[:, ac * S_c + st : ac * S_c + st + blocks[blk]],
                            start=(ac == 0),
                            stop=(ac == NAC - 1),
                        )
                nc.scalar.activation(
                    out=e_b[:, :S_c], in_=sc_ps[:, :S_c], func=AF.Exp
                )
                nc.sync.dma_start(out=eo[bi][:, :S_c], in_=e_b[0:1, :S_c])

                # weighted sum: num[d] = sum_s xt[d, s] * e[s], one fused
                # affine_mul_reduce per d-chunk (f32 accumulate). For the
                # last batch, route two chunks through ScalarE Copy+accum
                # so the pipeline drain runs on two engines.
                for dc in range(NDC):
                    prod = prodp.tile([128, S_cmax], bf16, tag="prod")
                    col = bi * NDC + dc
                    if bi == BL - 1 and dc >= 2:
                        nc.vector.tensor_mul(
                            out=prod[:, :S_c],
                            in0=xt_t[:, dc * S_c : dc * S_c + S_c],
                            in1=e_b[:, :S_c],
                        )
                        dump = dumpp.tile([128, S_cmax], bf16, tag="dump")
                        nc.scalar.activation(
                            out=dump[:, :S_c],
                            in_=prod[:, :S_c],
                            func=AF.Copy,
                            accum_out=num_sb[:, col : col + 1],
                        )
                    else:
                        nc.vector.affine_mul_reduce(
                            out=prod[:, :S_c],
                            accum_out=num_sb[:, col : col + 1],
                            in0=xt_t[:, dc * S_c : dc * S_c + S_c],
                            in1=e_b[:, :S_c],
                            scale=1.0,
                            bias=0.0,
                        )

            nc.sync.dma_start(out=num[:, :], in_=num_sb)

    nc.finalize()
    return nc


def _get_nc(sc_list):
    key = tuple(sc_list)
    if key not in _cache:
        _cache[key] = _build_bass(sc_list)
    return _cache[key]


def _prepare(x, mask, W, b, u):
    bf = ml_dtypes.bfloat16
    x = np.asarray(x, dtype=np.float32)
    mask = np.asarray(mask).astype(bool)

    counts = mask.sum(axis=1)

    # sort batches by count (desc); batch perm[j*NCORES + c] -> core c, slot j.
    # Slot j then needs only the max count within its band, rounded to 64.
    perm = np.argsort(-counts, kind="stable")
    sc_list = []
    for j in range(BL):
        band = counts[perm[j * NCORES : (j + 1) * NCORES]]
        mx = int(band.max())
        sc_list.append(min(S, max(256, 64 * ((mx + 63) // 64))))
    S_cmax = max(sc_list)

    # host-side compaction into the jagged packed layout:
    # xt_h[bi_slot, p, dc*S_c[j] + s] = x[batch, s_unmasked, dc*128 + p]
    xt_h = np.zeros((B, 128, NDC * S_cmax), dtype=bf)
    maskc = np.zeros((B, S_cmax), dtype=np.float32)
    for j in range(BL):
        S_c = sc_list[j]
        for c in range(NCORES):
            bidx = int(perm[j * NCORES + c])
            idx = np.flatnonzero(mask[bidx])
            xcb = np.zeros((S_c, D), dtype=np.float32)
            xcb[: idx.size] = x[bidx, idx]
            # [S_c, D] -> [128, NDC*S_c] with layout p, dc*S_c + s
            packed = (
                xcb.T.reshape(NDC, 128, S_c).transpose(1, 0, 2).reshape(128, NDC * S_c)
            )
            xt_h[c * BL + j, :, : NDC * S_c] = packed.astype(bf)
            maskc[c * BL + j, : idx.size] = 1.0

    w2_h = np.ascontiguousarray(
        np.asarray(W, dtype=np.float32).reshape(NDC, 128, A).transpose(1, 0, 2).reshape(128, NDC * A)
    ).astype(bf)
    u_col = np.asarray(u, dtype=np.float32)[:, 0].reshape(NAC, 128).T  # [128, NAC]
    u2_h = np.ascontiguousarray(
        np.repeat(u_col[:, :, None], 128, axis=2).reshape(128, NAC * 128)
    ).astype(bf)
    b2_h = np.ascontiguousarray(
        np.asarray(b, dtype=np.float32).reshape(NAC, 128).T
    ).astype(np.float32)
    return sc_list, perm, xt_h, maskc, w2_h, u2_h, b2_h


def kernel(x, mask, W, b, u):
    global last_results
    from concourse.bass_utils import run_bass_kernel_spmd

    sc_list, perm, xt_h, maskc, w2_h, u2_h, b2_h = _prepare(x, mask, W, b, u)
    nc = _get_nc(sc_list)
    in_maps = []
    for c in range(NCORES):
        sl = slice(c * BL, (c + 1) * BL)
        in_maps.append(
            {
                "xt": xt_h[sl],
                "w2": w2_h,
                "u2": u2_h,
                "b2": b2_h,
            }
        )

    try:
        res = run_bass_kernel_spmd(nc, in_maps, core_ids=list(range(NCORES)))
    except ModuleNotFoundError:
        # BASS_TRACE requested but the axon NTFF hook module is absent;
        # rerun without tracing.
        os.environ["BASS_NEVER_TRACE"] = "1"
        res = run_bass_kernel_spmd(nc, in_maps, core_ids=list(range(NCORES)))
    last_results = res

    out = np.empty((B, D), dtype=np.float32)
    for c in range(NCORES):
        num_h = res.results[c]["num"]                    # [128, BL*NDC] f32
        e_h = res.results[c]["eo"].astype(np.float32)    # [BL, 1, S_cmax]
        num_bd = (
            num_h.reshape(128, BL, NDC).transpose(1, 2, 0).reshape(BL, D)
        )
        for j in range(BL):
            bidx = int(perm[j * NCORES + c])
            sc = sc_list[j]
            den = (e_h[j, 0, :sc] * maskc[c * BL + j, :sc]).sum() + np.float32(EPS)
            out[bidx] = num_bd[j] / den
    return out.astype(np.float32)


# revision 28
# speedup vs baseline: 2.3466x; 1.0164x over previous
"""Trainium2 Bass kernel for attention-pooling (AttLayer).

Computes, per batch row b:
    z   = x[b] @ W + bias            # [S, A]
    t   = tanh(z)
    sc  = t @ u                      # [S]
    e   = exp(sc) * mask[b]
    out = (x[b]^T @ e) / (sum(e) + 1e-7)   # [D]

Sharding: data-parallel over batch across 8 NeuronCores (8 rows each).

Design (v5):
- Host compacts unmasked positions per row (~50% dense mask) and zero-pads.
  Padding rows of x are zero, so they contribute nothing to the numerator;
  the denominator is computed on host from the returned e row with the
  compacted mask. No mask work on device at all.
- Jagged slots: batches are sorted by unmasked count and distributed so
  slot j holds similar-count batches on every core. Each slot gets its own
  compacted length S_c[j] (64-multiple), so most slots run with 1024
  columns (two clean 512-wide matmul blocks, no tail) instead of the
  global max. One compile per slot-length tuple.
- x is streamed ONCE per batch in a transposed layout xt[p, dc*S_c+s] =
  x[s, dc*128+p], host-packed so each SBUF partition line is one
  contiguous DMA descriptor.
- Stage 1 (weight-major): z^T accumulated in a merged [128, <=1024] psum
  tile plus optional tail bank; tanh with per-partition bias runs as two
  big ScalarE instructions per a-chunk.
- Stage 2 uses a column-replicated u as lhsT (m=128, same cycle count as
  m=1), so exp(score) lands broadcast across all 128 partitions in one
  merged psum tile; a single Exp per batch.
- Weighted sum: tensor_mul on VectorE (bf16 2x) per d-chunk; reduction via
  a short fold tree on VectorE for 3 chunks and a Copy+accum_out on
  ScalarE for the 4th, balancing engine load.
- Host: out = num / (sum(e * maskc) + EPS), un-permuted.
"""

import math
import os
import numpy as np
import ml_dtypes

B, S, D, A = 64, 2048, 512, 256
NCORES = 8
BL = B // NCORES          # batches per core
NDC = D // 128            # 4 d-chunks
NAC = A // 128            # 2 a-chunks
EPS = 1e-7

_cache = {}
last_results = None       # BassKernelResults of the most recent run


def _blocks_of(S_c):
    """Split S_c into seq blocks of at most 512."""
    out = []
    rem = S_c
    while rem > 0:
        blk = min(512, rem)
        out.append(blk)
        rem -= blk
    return out


def _build_bass(sc_list):
    import concourse.mybir as mybir
    import concourse.tile as tile
    from concourse import bacc

    f32 = mybir.dt.float32
    bf16 = mybir.dt.bfloat16
    AF = mybir.ActivationFunctionType

    assert len(sc_list) == BL
    S_cmax = max(sc_list)
    assert all(sc % 64 == 0 for sc in sc_list)

    nc = bacc.Bacc()

    xt = nc.declare_dram_parameter("xt", [BL, 128, NDC * S_cmax], bf16, isOutput=False)
    w2 = nc.declare_dram_parameter("w2", [128, NDC * A], bf16, isOutput=False)
    u2 = nc.declare_dram_parameter("u2", [128, NAC * 128], bf16, isOutput=False)
    b2 = nc.declare_dram_parameter("b2", [128, NAC], f32, isOutput=False)
    num = nc.declare_dram_parameter("num", [128, BL * NDC], f32, isOutput=True)
    eo = nc.declare_dram_parameter("eo", [BL, 1, S_cmax], bf16, isOutput=True)

    with tile.TileContext(nc) as tc:
        with (
            tc.tile_pool(name="consts", bufs=1) as consts,
            tc.tile_pool(name="xtp", bufs=4) as xtp,
            tc.tile_pool(name="ttp", bufs=2) as ttp,
            tc.tile_pool(name="ebp", bufs=2) as ebp,
            tc.tile_pool(name="prodp", bufs=3) as prodp,
            tc.tile_pool(name="foldp", bufs=3) as foldp,
            tc.tile_pool(name="dumpp", bufs=2) as dumpp,
            tc.tile_pool(name="pt", bufs=2, space="PSUM") as pt,
            tc.tile_pool(name="ptt", bufs=1, space="PSUM") as ptt,
            tc.tile_pool(name="psc", bufs=1, space="PSUM") as psc,
        ):
            # --- constants; u/b issued after the first xt chunk so they
            # don't delay stage 1 of batch 0 ---
            w_sb = consts.tile([128, NDC * A], bf16)
            nc.sync.dma_start(out=w_sb, in_=w2[:, :])
            u_sb = consts.tile([128, NAC * 128], bf16)
            b_sb = consts.tile([128, NAC], f32)
            num_sb = consts.tile([128, BL * NDC], f32)

            for bi in range(BL):
                S_c = sc_list[bi]
                blocks = _blocks_of(S_c)
                NBLK = len(blocks)
                starts = [sum(blocks[:i]) for i in range(NBLK)]
                S_main = min(S_c, 1024)
                has_tail = S_main < S_c

                # per-d-chunk DMAs: stage 1 starts after 1/4 of the batch
                xt_t = xtp.tile([128, NDC * S_cmax], bf16)
                for dc in range(NDC):
                    nc.sync.dma_start(
                        out=xt_t[:, dc * S_c : (dc + 1) * S_c],
                        in_=xt[bi][:, dc * S_c : (dc + 1) * S_c],
                    )
                if bi == 0:
                    nc.sync.dma_start(out=u_sb, in_=u2[:, :])
                    nc.sync.dma_start(out=b_sb, in_=b2[:, :])

                # stage 1 (weight-major): z^T[a, s] = W^T @ x^T, then tanh.
                tt = ttp.tile([128, NAC * S_cmax], bf16)
                for ac in range(NAC):
                    ps_big = pt.tile([128, 1024], f32, tag="pst", name="ps_big")
                    ps_tail = (
                        ptt.tile([128, 128], f32, tag="ptt", name="ps_tail")
                        if has_tail
                        else None
                    )
                    for dc in range(NDC):
                        lo = dc * A + ac * 128
                        for blk in range(NBLK):
                            st = starts[blk]
                            out_ps = (
                                ps_big[:, st : st + blocks[blk]]
                                if st < S_main
                                else ps_tail[:, : blocks[blk]]
                            )
                            nc.tensor.matmul(
                                out=out_ps,
                                lhsT=w_sb[:, lo : lo + 128],
                                rhs=xt_t[
                                    :,
                                    dc * S_c + st : dc * S_c + st + blocks[blk],
                                ],
                                start=(dc == 0),
                                stop=(dc == NDC - 1),
                            )
                    nc.scalar.activation(
                        out=tt[:, ac * S_c : ac * S_c + S_main],
                        in_=ps_big[:, :S_main],
                        func=AF.Tanh,
                        bias=b_sb[:, ac : ac + 1],
                        scale=1.0,
                    )
                    if has_tail:
                        nc.scalar.activation(
                            out=tt[:, ac * S_c + S_main : ac * S_c + S_c],
                            in_=ps_tail[:, : S_c - S_main],
                            func=AF.Tanh,
                            bias=b_sb[:, ac : ac + 1],
                            scale=1.0,
                        )

                # stage 2: score broadcast across partitions via replicated u,
                # one merged psum tile, one Exp -> e_bcast [128, S_c]
                e_b = ebp.tile([128, S_cmax], bf16)
                sc_ps = psc.tile([128, 1152], f32, tag="psc", name="sc_ps")
                for ac in range(NAC):
                    for blk in range(NBLK):
                        st = starts[blk]
                        nc.tensor.matmul(
                            out=sc_ps[:, st : st + blocks[blk]],
                            lhsT=u_sb[:, ac * 128 : (ac + 1) * 128],
                            rhs=tt[:, ac * S_c + st : ac * S_c + st + blocks[blk]],
                            start=(ac == 0),
                            stop=(ac == NAC - 1),
                        )
                nc.scalar.activation(
                    out=e_b[:, :S_c], in_=sc_ps[:, :S_c], func=AF.Exp
                )
                nc.sync.dma_start(out=eo[bi][:, :S_c], in_=e_b[0:1, :S_c])

                # weighted sum: num[d] = sum_s xt[d, s] * e[s], one fused
                # affine_mul_reduce per d-chunk (f32 accumulate). For the
                # last batch, route two chunks through ScalarE Copy+accum
                # so the pipeline drain runs on two engines.
                for dc in range(NDC):
                    prod = prodp.tile([128, S_cmax], bf16, tag="prod")
                    col = bi * NDC + dc
                    if bi == BL - 1 and dc >= 2:
                        nc.vector.tensor_mul(
                            out=prod[:, :S_c],
                            in0=xt_t[:, dc * S_c : dc * S_c + S_c],
                            in1=e_b[:, :S_c],
                        )
                        dump = dumpp.tile([128, S_cmax], bf16, tag="dump")
                        nc.scalar.activation(
                            out=dump[:, :S_c],
                            in_=prod[:, :S_c],
                            func=AF.Copy,
                            accum_out=num_sb[:, col : col + 1],
                        )
                    else:
                        nc.vector.affine_mul_reduce(
                            out=prod[:, :S_c],
                            accum_out=num_sb[:, col : col + 1],
                            in0=xt_t[:, dc * S_c : dc * S_c + S_c],
                            in1=e_b[:, :S_c],
                            scale=1.0,
                            bias=0.0,
                        )

            nc.sync.dma_start(out=num[:, :], in_=num_sb)

    nc.finalize()
    return nc


def _get_nc(sc_list):
    key = tuple(sc_list)
    if key not in _cache:
        _cache[key] = _build_bass(sc_list)
    return _cache[key]


def _prepare(x, mask, W, b, u):
    bf = ml_dtypes.bfloat16
    x = np.asarray(x, dtype=np.float32)
    mask = np.asarray(mask).astype(bool)

    counts = mask.sum(axis=1)

    # sort batches by count (desc); batch perm[j*NCORES + c] -> core c, slot j.
    # Slot j then needs only the max count within its band, rounded to 64.
    perm = np.argsort(-counts, kind="stable")
    sc_list = []
    for j in range(BL):
        band = counts[perm[j * NCORES : (j + 1) * NCORES]]
        mx = int(band.max())
        sc_list.append(min(S, max(256, 64 * ((mx + 63) // 64))))
    S_cmax = max(sc_list)

    # host-side compaction into the jagged packed layout:
    # xt_h[bi_slot, p, dc*S_c[j] + s] = x[batch, s_unmasked, dc*128 + p]
    xt_h = np.zeros((B, 128, NDC * S_cmax), dtype=bf)
    maskc = np.zeros((B, S_cmax), dtype=np.float32)
    for j in range(BL):
        S_c = sc_list[j]
        for c in range(NCORES):
            bidx = int(perm[j * NCORES + c])
            idx = np.flatnonzero(mask[bidx])
            xcb = np.zeros((S_c, D), dtype=np.float32)
            xcb[: idx.size] = x[bidx, idx]
            # [S_c, D] -> [128, NDC*S_c] with layout p, dc*S_c + s
            packed = (
                xcb.T.reshape(NDC, 128, S_c).transpose(1, 0, 2).reshape(128, NDC * S_c)
            )
            xt_h[c * BL + j, :, : NDC * S_c] = packed.astype(bf)
            maskc[c * BL + j, : idx.size] = 1.0

    w2_h = np.ascontiguousarray(
        np.asarray(W, dtype=np.float32).reshape(NDC, 128, A).transpose(1, 0, 2).reshape(128, NDC * A)
    ).astype(bf)
    u_col = np.asarray(u, dtype=np.float32)[:, 0].reshape(NAC, 128).T  # [128, NAC]
    u2_h = np.ascontiguousarray(
        np.repeat(u_col[:, :, None], 128, axis=2).reshape(128, NAC * 128)
    ).astype(bf)
    b2_h = np.ascontiguousarray(
        np.asarray(b, dtype=np.float32).reshape(NAC, 128).T
    ).astype(np.float32)
    return sc_list, perm, xt_h, maskc, w2_h, u2_h, b2_h


def kernel(x, mask, W, b, u):
    global last_results
    from concourse.bass_utils import run_bass_kernel_spmd

    sc_list, perm, xt_h, maskc, w2_h, u2_h, b2_h = _prepare(x, mask, W, b, u)
    nc = _get_nc(sc_list)
    in_maps = []
    for c in range(NCORES):
        sl = slice(c * BL, (c + 1) * BL)
        in_maps.append(
            {
                "xt": xt_h[sl],
                "w2": w2_h,
                "u2": u2_h,
                "b2": b2_h,
            }
        )

    try:
        res = run_bass_kernel_spmd(nc, in_maps, core_ids=list(range(NCORES)))
    except ModuleNotFoundError:
        # BASS_TRACE requested but the axon NTFF hook module is absent;
        # rerun without tracing.
        os.environ["BASS_NEVER_TRACE"] = "1"
        res = run_bass_kernel_spmd(nc, in_maps, core_ids=list(range(NCORES)))
    last_results = res

    out = np.empty((B, D), dtype=np.float32)
    for c in range(NCORES):
        num_h = res.results[c]["num"]                    # [128, BL*NDC] f32
        e_h = res.results[c]["eo"].astype(np.float32)    # [BL, 1, S_cmax]
        num_bd = (
            num_h.reshape(128, BL, NDC).transpose(1, 2, 0).reshape(BL, D)
        )
        for j in range(BL):
            bidx = int(perm[j * NCORES + c])
            sc = sc_list[j]
            den = (e_h[j, 0, :sc] * maskc[c * BL + j, :sc]).sum() + np.float32(EPS)
            out[bidx] = num_bd[j] / den
    return out.astype(np.float32)
